# revision 1
# baseline (speedup 1.0000x reference)
"""LRU forward on 8 Trainium2 NeuronCores.

Sharding: 8 shards = 4 batches x 2 sequence halves (L_local = 2048).
Per-core dataflow is fully transposed (d_model on SBUF partitions, time on
the free dim):

  input proj   Bu^T = Bg_cat^T @ x^T as fp8e4 DoubleRow matmuls (weights
               pre-scaled x64 on the host to stay in e4m3 normal range; the
               1/64 is folded into the PSUM->SBUF downcast scale)
  scan         complex diagonal recurrence -> rotating frame e^{-i theta t}
               turns it into 4 real per-lane scans (hardware
               tensor_tensor_scan; fp32 multiplier + internal state, fp16
               data).  Carry between sequence halves exchanged with a
               pairwise AllReduce and applied as g += r^{512m+s+1} * c_hat
               (decay table has the per-chunk factor folded in); chunk 0's
               rot-out runs carry-free inside the exchange bubble and is
               corrected afterwards via host P/Q tables (rpow x cos/sin)
  output proj  ys^T = CT_cat^T @ h in fp16, plus the D*x skip path as one
               fp8 DoubleRow matmul per block: diag(D_hi) against the x_hi
               plane and a host-merged plane xm = x_lo + (D_lo/D_hi)*x_hi
               (reproduces D*x to ~1%)

Rotation elementwise work runs on DVE in fp16 (2x perf mode) with the c=1
imaginary path offloaded to gpsimd; PSUM->SBUF downcasts run on the scalar
(Activation) engine; tables are SBUF resident and loaded once.  Phase A uses
non-uniform time chunks (small at both ends) to shorten pipeline fill and
the scan->collective tail, and junk matmuls chained on the exchange arrival
keep the tensor engine's p-state ramped through the collective bubble.
Host side only preprocesses/shards and reassembles the output.
"""

import os

import numpy as np

B, L, D, N = 4, 4096, 1024, 256
NCORE = 8
LLOC = L // 2          # per-core sequence length
MC = 512               # time chunk (matmul moving free dim)
NMC = LLOC // MC       # 4 chunks
N2 = 2 * N             # stacked re|im channels

_CACHE = {}
LAST_RESULTS = None    # test.py reads exec_time_ns from here


def _build():
    import concourse.bass as bass
    import concourse.mybir as mybir
    import concourse.tile as tile
    from concourse import bacc

    f32 = mybir.dt.float32
    f16 = mybir.dt.float16
    f8 = mybir.dt.float8e4
    DR = mybir.MatmulPerfMode.DoubleRow
    ADD = mybir.AluOpType.add
    SUB = mybir.AluOpType.subtract
    MUL = mybir.AluOpType.mult

    nc = bacc.Bacc("TRN2", target_bir_lowering=False, debug=False, num_devices=NCORE)

    # ---- DRAM I/O (per-core) ----
    # xq holds two fp8 planes of x^T: [:, 0] = fp8(x) and [:, 1] = the
    # merged residual plane xm = (x - hi) + (D_lo/D_hi)*hi.  The input
    # projection reads the hi plane; the skip path contracts both against
    # duplicated diag(D_hi) blocks in a single DoubleRow matmul.
    xqd = nc.dram_tensor("xq", [128, 2, 8, LLOC], f8, kind="ExternalInput").ap()
    bg8d = nc.dram_tensor("bg8", [128, 8, N2], f8, kind="ExternalInput").ap()
    ctd = nc.dram_tensor("ct", [128, 4, D], f16, kind="ExternalInput").ap()
    ddd = nc.dram_tensor("ddiag", [128, 8, 2, 128], f8, kind="ExternalInput").ap()
    csd = nc.dram_tensor("cost", [128, 2, LLOC], f16, kind="ExternalInput").ap()
    snd = nc.dram_tensor("sint", [128, 2, LLOC], f16, kind="ExternalInput").ap()
    rbd = nc.dram_tensor("rb", [128, 2, MC], f32, kind="ExternalInput").ap()
    rpwd = nc.dram_tensor("rpow", [128, 2, NMC, MC], f16, kind="ExternalInput").ap()
    pcd = nc.dram_tensor("pctab", [128, 2, MC], f16, kind="ExternalInput").ap()
    qcd = nc.dram_tensor("qctab", [128, 2, MC], f16, kind="ExternalInput").ap()
    r48d = nc.dram_tensor("rot48", [128, 2, 3], f32, kind="ExternalInput").ap()
    gmd = nc.dram_tensor("gmask", [128, 4], f32, kind="ExternalInput").ap()
    pmd = nc.dram_tensor("pmask", [128, 4], f32, kind="ExternalInput").ap()
    outd = nc.dram_tensor("outT", [128, 8, LLOC], f16, kind="ExternalOutput").ap()

    with tile.TileContext(nc) as tc:
        from contextlib import ExitStack

        with ExitStack() as st:
            cpool = st.enter_context(tc.tile_pool(name="consts", bufs=1))
            xpool = st.enter_context(tc.tile_pool(name="xt", bufs=1))
            gpool = st.enter_context(tc.tile_pool(name="g", bufs=1))
            bpool = st.enter_context(tc.tile_pool(name="bu", bufs=3))
            upool = st.enter_context(tc.tile_pool(name="u", bufs=3))
            hpool = st.enter_context(tc.tile_pool(name="h", bufs=3))
            opool = st.enter_context(tc.tile_pool(name="o", bufs=3))
            ps = st.enter_context(tc.tile_pool(name="ps", bufs=2, space="PSUM"))
            dram = st.enter_context(tc.tile_pool(name="dram", bufs=1, space="DRAM"))

            # ---- SBUF residents; DMA order is pipeline priority order ----
            bg8_sb = cpool.tile([128, 8, N2], f8, tag="bg8", name="bg8")
            xq_sb = xpool.tile([128, 2, 8, LLOC], f8, tag="xq", name="xq")
            cs_sb = cpool.tile([128, 2, LLOC], f16, tag="cs", name="cs")
            sn_sb = cpool.tile([128, 2, LLOC], f16, tag="sn", name="sn")
            rb_sb = cpool.tile([128, 2, MC], f32, tag="rb", name="rb")
            ct_sb = cpool.tile([128, 4, D], f16, tag="ct", name="ct")
            dd_sb = cpool.tile([128, 8, 2, 128], f8, tag="dd", name="dd")
            rpw_sb = cpool.tile([128, 2, NMC, MC], f16, tag="rpw", name="rpw")
            pc_sb = cpool.tile([128, 2, MC], f16, tag="pc", name="pc")
            qc_sb = cpool.tile([128, 2, MC], f16, tag="qc", name="qc")
            r48_sb = cpool.tile([128, 2, 3], f32, tag="r48", name="r48")
            gm_sb = cpool.tile([128, 4], f32, tag="gm", name="gm")
            pm_sb = cpool.tile([128, 4], f32, tag="pm", name="pm")

            # phase-A chunking: small chunks at the ends to shorten pipeline
            # fill and the scan->collective tail latency
            CHA = [(0, 256), (256, 256), (512, 512), (1024, 512),
                   (1536, 384), (1920, 128)]

            nc.sync.dma_start(bg8_sb[:, 0:4, :], bg8d[:, 0:4, :])
            nc.sync.dma_start(xq_sb[:, 0, :, 0:256], xqd[:, 0, :, 0:256])
            nc.sync.dma_start(bg8_sb[:, 4:8, :], bg8d[:, 4:8, :])
            nc.sync.dma_start(xq_sb[:, 0, :, 256:MC], xqd[:, 0, :, 256:MC])
            nc.sync.dma_start(cs_sb[:, :, 0:2 * MC], csd[:, :, 0:2 * MC])
            nc.sync.dma_start(sn_sb[:, :, 0:2 * MC], snd[:, :, 0:2 * MC])
            nc.sync.dma_start(xq_sb[:, 0, :, MC:2 * MC], xqd[:, 0, :, MC:2 * MC])
            nc.sync.dma_start(rb_sb[:], rbd[:, :, :])
            nc.sync.dma_start(xq_sb[:, 0, :, 2 * MC:3 * MC],
                              xqd[:, 0, :, 2 * MC:3 * MC])
            nc.sync.dma_start(cs_sb[:, :, 2 * MC:], csd[:, :, 2 * MC:])
            nc.sync.dma_start(xq_sb[:, 0, :, 3 * MC:], xqd[:, 0, :, 3 * MC:])
            nc.sync.dma_start(sn_sb[:, :, 2 * MC:], snd[:, :, 2 * MC:])
            nc.sync.dma_start(dd_sb[:], ddd[:, :, :, :])
            nc.sync.dma_start(pc_sb[:], pcd[:, :, :])
            nc.sync.dma_start(qc_sb[:], qcd[:, :, :])
            nc.sync.dma_start(ct_sb[:], ctd[:, :, :])
            nc.sync.dma_start(rpw_sb[:], rpwd[:, :, :, :])
            nc.sync.dma_start(r48_sb[:], r48d[:, :, :])
            nc.sync.dma_start(gm_sb[:], gmd[:, :])
            nc.sync.dma_start(pm_sb[:], pmd[:, :])
            # x merged-plane loads late: its consumers (skip matmuls) run
            # inside the collective bubble, and an early load would displace
            # phase-A-critical transfers on the DMA engines
            nc.sync.dma_start(xq_sb[:, 1, :, :], xqd[:, 1, :, :])

            g4 = []
            for tt_ in range(4):
                g4.append(gpool.tile([128, LLOC], f16, tag=f"g{tt_}", name=f"g{tt_}"))

            # ---- phase A: input projection + rot-in + scan, per time chunk ----
            # input proj: fp8 DoubleRow matmuls (weights pre-scaled x64 on the
            # host; the 1/64 is folded into the PSUM->SBUF downcast scale).
            # rot-in: the c=1 imaginary path runs on gpsimd to unload DVE.
            for m, (st, w) in enumerate(CHA):
                ms = slice(st, st + w)
                bu = bpool.tile([128, 4, MC], f16, tag="bu", name=f"bu_{m}")
                for j in range(4):
                    pt = ps.tile([128, MC], f32, tag=f"p{j}", name=f"pbu{j}_{m}")
                    for k2 in range(4):
                        nc.tensor.matmul(
                            pt[:, 0:w],
                            bg8_sb[:, 2 * k2:2 * k2 + 2, 128 * j:128 * (j + 1)],
                            xq_sb[:, 0, 2 * k2:2 * k2 + 2, ms],
                            start=(k2 == 0),
                            stop=(k2 == 3),
                            perf_mode=DR,
                        )
                    nc.scalar.mul(bu[:, j, 0:w], pt[:, 0:w], 1.0 / 64.0)
                # rot-in as paired 2-free-dim ops: each instruction covers
                # both complex groups ([128, 2, w] slices of the stacked bu
                # tile against cs/sn[:, 0:2, ms]); the two cos/sin products
                # feeding u_im run on gpsimd to balance the conveyor
                csp = cs_sb[:, 0:2, ms]
                snp = sn_sb[:, 0:2, ms]
                p1 = upool.tile([128, 2, MC], f16, tag="p1", name=f"p1_{m}")
                nc.vector.tensor_tensor(p1[:, :, 0:w], bu[:, 0:2, 0:w], csp, MUL)
                p4 = upool.tile([128, 2, MC], f16, tag="p4", name=f"p4_{m}")
                nc.gpsimd.tensor_tensor(p4[:, :, 0:w], bu[:, 0:2, 0:w], snp, MUL)
                p2 = upool.tile([128, 2, MC], f16, tag="p2", name=f"p2_{m}")
                nc.vector.tensor_tensor(p2[:, :, 0:w], bu[:, 2:4, 0:w], snp, MUL)
                p3 = upool.tile([128, 2, MC], f16, tag="p3", name=f"p3_{m}")
                nc.gpsimd.tensor_tensor(p3[:, :, 0:w], bu[:, 2:4, 0:w], csp, MUL)
                u_re = upool.tile([128, 2, MC], f16, tag="ure", name=f"ure_{m}")
                nc.vector.tensor_tensor(u_re[:, :, 0:w], p1[:, :, 0:w],
                                        p2[:, :, 0:w], ADD)
                u_im = upool.tile([128, 2, MC], f16, tag="uim", name=f"uim_{m}")
                nc.vector.tensor_tensor(u_im[:, :, 0:w], p3[:, :, 0:w],
                                        p4[:, :, 0:w], SUB)
                u4 = [u_re[:, 0, :], u_re[:, 1, :], u_im[:, 0, :], u_im[:, 1, :]]
                for tt_ in range(4):
                    init = 0.0 if m == 0 else g4[tt_][:, st - 1:st]
                    nc.vector.tensor_tensor_scan(
                        g4[tt_][:, ms], rb_sb[:, tt_ & 1, 0:w],
                        u4[tt_][:, 0:w], init, MUL, ADD)

            # ---- phase B: carry exchange (pairwise AllReduce) ----
            stage = cpool.tile([128, 4], f32, tag="stage", name="stage")
            for tt_ in range(4):
                nc.vector.tensor_copy(stage[:, tt_:tt_ + 1], g4[tt_][:, LLOC - 1:LLOC])
            # scatter my carry into my pair's 4-column group (zero elsewhere)
            stage16 = cpool.tile([128, 16], f32, tag="stage16", name="stage16")
            for p in range(4):
                nc.vector.tensor_scalar_mul(
                    stage16[:, 4 * p:4 * (p + 1)], stage[:], gm_sb[:, p:p + 1])
            in_cc = dram.tile([128, 16], f32, tag="incc", name="incc")
            out_cc = dram.tile([128, 16], f32, tag="outcc", name="outcc",
                               addr_space="Shared")
            nc.sync.dma_start(in_cc[:], stage16[:])
            if os.environ.get("LRU_NOCC", "0") == "1":
                # collective-free variant for TimelineSim bottleneck analysis
                nc.sync.dma_start(out_cc[:], in_cc[:])
            else:
                nc.gpsimd.collective_compute(
                    "AllReduce",
                    mybir.AluOpType.add,
                    replica_groups=[list(range(NCORE))],
                    ins=[in_cc.opt()],
                    outs=[out_cc.opt()],
                )
            recv16 = cpool.tile([128, 16], f32, tag="recv16", name="recv16")
            nc.sync.dma_start(recv16[:], out_cc[:])


            # skip-path matmuls for chunk 0 run inside the collective bubble
            m0 = slice(0, MC)
            pre_ps = []
            with tc.high_priority():
                for di in range(8):
                    pt = ps.tile([128, MC], f32, tag=f"p{di % 4}", name=f"o{di}_0")
                    if di > 0:
                        nc.tensor.matmul(pt[:], dd_sb[:, di, :, :],
                                         xq_sb[:, :, di, m0],
                                         start=True, stop=False, perf_mode=DR)
                    pre_ps.append(pt)

            # PE warm-up chained on the exchange arrival: junk matmuls into a
            # corner of pre_ps[0] (its real skip matmul below uses start=True
            # and overwrites) keep the tensor engine's p-state ramped through
            # the bubble so the first output matmuls run at full clock
            jsA = cpool.tile([128, 16], f16, tag="jsA", name="jsA")
            nc.scalar.copy(jsA[:], stage16[:])
            for _ in range(34):
                nc.tensor.matmul(pre_ps[0][0:16, 0:128], jsA[:, 0:16],
                                 cs_sb[:, 0, 0:128], start=True, stop=True)
            jsrc = cpool.tile([128, 16], f16, tag="jsrc", name="jsrc")
            nc.scalar.copy(jsrc[:], recv16[:])
            for _ in range(50):
                nc.tensor.matmul(pre_ps[0][0:16, 0:128], jsrc[:, 0:16],
                                 cs_sb[:, 0, 0:128], start=True, stop=True)
            nc.tensor.matmul(pre_ps[0][:], dd_sb[:, 0, :, :],
                             xq_sb[:, :, 0, m0],
                             start=True, stop=False, perf_mode=DR)

            # chunk-0 carry-free rot-out also runs inside the bubble; the
            # carry correction lands later via the P/Q tables (rpow folded
            # with cos/sin on the host) and per-lane chat scalars.
            # wait-hint: schedule these AFTER the exchange staging ops above
            # (their data deps are ready much earlier and the scheduler would
            # otherwise run them first, delaying the exchange).
            tc.tile_set_cur_wait(0.040)
            h4_0 = [None] * 4
            for c in range(2):
                csm = cs_sb[:, c, m0]
                snm = sn_sb[:, c, m0]
                t1 = hpool.tile([128, MC], f16, tag="tmpC", name=f"tE{c}_b")
                nc.vector.tensor_tensor(t1[:], g4[c][:, m0], csm, MUL)
                t2 = hpool.tile([128, MC], f16, tag="tmpD", name=f"tF{c}_b")
                nc.vector.tensor_tensor(t2[:], g4[2 + c][:, m0], snm, MUL)
                h_re = hpool.tile([128, MC], f16, tag=f"h{c}", name=f"h{c}_b")
                nc.vector.tensor_tensor(h_re[:], t1[:], t2[:], SUB)
                h4_0[c] = h_re
                t3 = hpool.tile([128, MC], f16, tag="tmpC", name=f"tG{c}_b")
                nc.vector.tensor_tensor(t3[:], g4[2 + c][:, m0], csm, MUL)
                t4 = hpool.tile([128, MC], f16, tag="tmpD", name=f"tH{c}_b")
                nc.vector.tensor_tensor(t4[:], g4[c][:, m0], snm, MUL)
                h_im = hpool.tile([128, MC], f16, tag=f"h{2+c}", name=f"h{2+c}_b")
                nc.vector.tensor_tensor(h_im[:], t3[:], t4[:], ADD)
                h4_0[2 + c] = h_im

            # select my pair's group: recv = sum_p recv16[:, 4p:4p+4] * pm[:, p]
            recv = cpool.tile([128, 4], f32, tag="recv", name="recv")
            nc.vector.tensor_scalar_mul(recv[:], recv16[:, 0:4], pm_sb[:, 0:1])
            for p in range(1, 4):
                nc.vector.scalar_tensor_tensor(
                    recv[:], recv16[:, 4 * p:4 * (p + 1)], pm_sb[:, p:p + 1],
                    recv[:], MUL, ADD)

            # chat = rot48 * recv (per-lane complex rotation), via STT pairs;
            # cols 4,5 hold -chat_im for the chunk-0 P/Q correction
            chat = cpool.tile([128, 6], f32, tag="chat", name="chat")
            tca = cpool.tile([128, 1], f32, tag="tca", name="tca")
            tcb = cpool.tile([128, 1], f32, tag="tcb", name="tcb")
            for c in range(2):
                c48 = r48_sb[:, c, 0:1]
                s48 = r48_sb[:, c, 1:2]
                ns48 = r48_sb[:, c, 2:3]
                # chat_re = recv_re*cos48 - recv_im*sin48
                nc.vector.tensor_tensor(tca[:], recv[:, c:c + 1], c48, MUL)
                nc.vector.scalar_tensor_tensor(
                    chat[:, c:c + 1], recv[:, 2 + c:3 + c], ns48, tca[:], MUL, ADD)
                # chat_im = recv_im*cos48 + recv_re*sin48
                nc.vector.tensor_tensor(tcb[:], recv[:, 2 + c:3 + c], c48, MUL)
                nc.vector.scalar_tensor_tensor(
                    chat[:, 2 + c:3 + c], recv[:, c:c + 1], s48, tcb[:], MUL, ADD)
                nc.vector.tensor_scalar_mul(
                    chat[:, 4 + c:5 + c], chat[:, 2 + c:3 + c], -1.0)

            # ---- phase C: carry fix + rot-out + output projection ----
            # h4 index: 0 = re(c0), 1 = re(c1), 2 = im(c0), 3 = im(c1); the
            # output matmuls consume them in that order, so compute the re
            # components (and their carry fixes) first.
            # tile_wait_until: scheduler-only hint that phase C becomes ready
            # late (after the carry exchange) so ready-early work (the
            # skip-path matmuls) is ordered ahead of it in the engine streams.
            tc.tile_set_cur_wait(0.045)
            for m in range(NMC):
                ms = slice(m * MC, (m + 1) * MC)

                def fix(mm, tt_):
                    msf = slice(mm * MC, (mm + 1) * MC)
                    nc.vector.scalar_tensor_tensor(
                        g4[tt_][:, msf],
                        rpw_sb[:, tt_ & 1, mm, :],
                        chat[:, tt_:tt_ + 1],
                        g4[tt_][:, msf],
                        MUL,
                        ADD,
                    )

                if m == 0:
                    # apply the carry to the pre-computed local rot-out:
                    # h_re += chat_re*P - chat_im*Q ; h_im += chat_im*P + chat_re*Q
                    # chunk 1's carry fixes are interleaved between the
                    # correction pairs so its rot-out ladder starts before
                    # all of chunk 0's corrections retire on DVE
                    h4 = h4_0
                    for comp, pcol, qcol, pre in ((0, 0, 4, 0), (1, 1, 5, 2),
                                                  (2, 2, 0, 1), (3, 3, 1, 3)):
                        c = comp & 1
                        nc.vector.scalar_tensor_tensor(
                            h4[comp][:], pc_sb[:, c, :], chat[:, pcol:pcol + 1],
                            h4[comp][:], MUL, ADD)
                        nc.vector.scalar_tensor_tensor(
                            h4[comp][:], qc_sb[:, c, :], chat[:, qcol:qcol + 1],
                            h4[comp][:], MUL, ADD)
                        fix(1, pre)
                else:
                    # carry fix (STT only runs on DVE; the compiler rejects
                    # it on the gpsimd engine); chunk 1's were issued above
                    if m > 1:
                        for tt_ in (0, 2, 1, 3):
                            fix(m, tt_)
                    h4 = [None] * 4
                    for c in range(2):
                        csm = cs_sb[:, c, ms]
                        snm = sn_sb[:, c, ms]
                        t1 = hpool.tile([128, MC], f16, tag="tmpC", name=f"tE{c}_{m}")
                        nc.vector.tensor_tensor(t1[:], g4[c][:, ms], csm, MUL)
                        t2 = hpool.tile([128, MC], f16, tag="tmpD", name=f"tF{c}_{m}")
                        nc.vector.tensor_tensor(t2[:], g4[2 + c][:, ms], snm, MUL)
                        h_re = hpool.tile([128, MC], f16, tag=f"h{c}", name=f"h{c}_{m}")
                        nc.vector.tensor_tensor(h_re[:], t1[:], t2[:], SUB)
                        h4[c] = h_re
                    for c in range(2):
                        csm = cs_sb[:, c, ms]
                        snm = sn_sb[:, c, ms]
                        t3 = hpool.tile([128, MC], f16, tag="tmpC", name=f"tG{c}_{m}")
                        nc.vector.tensor_tensor(t3[:], g4[2 + c][:, ms], csm, MUL)
                        t4 = hpool.tile([128, MC], f16, tag="tmpD", name=f"tH{c}_{m}")
                        nc.vector.tensor_tensor(t4[:], g4[c][:, ms], snm, MUL)
                        h_im = hpool.tile([128, MC], f16, tag=f"h{2+c}",
                                          name=f"h{2+c}_{m}")
                        nc.vector.tensor_tensor(h_im[:], t3[:], t4[:], ADD)
                        h4[2 + c] = h_im
                ot = opool.tile([128, 8, MC], f16, tag="ot", name=f"ot_{m}")
                for di in range(8):
                    if m == 0:
                        pt = pre_ps[di]
                    else:
                        pt = ps.tile([128, MC], f32, tag=f"p{di % 4}",
                                     name=f"o{di}_{m}")
                        nc.tensor.matmul(pt[:], dd_sb[:, di, :, :],
                                         xq_sb[:, :, di, ms],
                                         start=True, stop=False, perf_mode=DR)
                    for tt_ in range(4):
                        nc.tensor.matmul(
                            pt[:],
                            ct_sb[:, tt_, 128 * di:128 * (di + 1)],
                            h4[tt_][:],
                            start=False,
                            stop=(tt_ == 3),
                        )
                    nc.scalar.copy(ot[:, di, :], pt[:])
                    if di == 3:
                        nc.sync.dma_start(outd[:, 0:4, ms], ot[:, 0:4, :])
                    elif m == NMC - 1 and di in (5, 6):
                        nc.sync.dma_start(outd[:, di - 1:di, ms],
                                          ot[:, di - 1:di, :])
                if m == NMC - 1:
                    nc.sync.dma_start(outd[:, 6:7, ms], ot[:, 6:7, :])
                    nc.sync.dma_start(outd[:, 7:8, ms], ot[:, 7:8, :])
                else:
                    nc.sync.dma_start(outd[:, 4:8, ms], ot[:, 4:8, :])

    nc.compile()
    return nc


def _prep(inputs):
    """Host-side parameter prep + sharding. Returns per-core input maps."""
    x = np.asarray(inputs["input_sequence"], np.float32)
    nu_log = np.asarray(inputs["nu_log"], np.float32)
    theta_log = np.asarray(inputs["theta_log"], np.float32)
    B_re = np.asarray(inputs["B_re"], np.float32)
    B_im = np.asarray(inputs["B_im"], np.float32)
    C_re = np.asarray(inputs["C_re"], np.float32)
    C_im = np.asarray(inputs["C_im"], np.float32)
    Dv = np.asarray(inputs["D"], np.float32)

    r32 = np.exp(-np.exp(nu_log, dtype=np.float32), dtype=np.float32)
    th = np.exp(theta_log, dtype=np.float32).astype(np.float64)
    gamma = np.sqrt((1.0 - r32 * r32).astype(np.float32))

    def blk(a, nb):  # [nb*128, F] -> [128, nb, F]
        return np.ascontiguousarray(
            a.reshape(nb, 128, a.shape[-1]).transpose(1, 0, 2))

    import ml_dtypes
    f8 = ml_dtypes.float8_e4m3

    bg = np.concatenate(
        [(gamma[:, None] * B_re).T, (gamma[:, None] * B_im).T], axis=1)  # [D, 512]
    ct = np.concatenate([C_re.T, -C_im.T], axis=0)                      # [512, D]
    # fp8 weights pre-scaled x64 to stay in e4m3 normal range; the matmul
    # consumer divides by 64 during the PSUM downcast
    bg83 = blk(bg * 64.0, 8).astype(f8)
    ct3 = blk(ct, 4).astype(np.float16)

    t = np.arange(LLOC, dtype=np.float64)
    ang = th[:, None] * t[None, :]
    cs3 = blk(np.cos(ang), 2).astype(np.float16)
    sn3 = blk(np.sin(ang), 2).astype(np.float16)
    r64 = r32.astype(np.float64)
    s = np.arange(MC, dtype=np.float64)
    # r^(512m + s + 1), chunk factor folded in
    rpw = (r64[:, None, None]
           ** (MC * np.arange(NMC, dtype=np.float64)[None, :, None]
               + s[None, None, :] + 1.0))                     # [N, NMC, MC]
    rpw3 = np.ascontiguousarray(
        rpw.reshape(2, 128, NMC, MC).transpose(1, 0, 2, 3)).astype(np.float16)
    # chunk-0 carry-correction tables: r^(s+1) * {cos,sin}(theta*s)
    pc = rpw[:, 0, :] * np.cos(ang[:, :MC])
    qc = rpw[:, 0, :] * np.sin(ang[:, :MC])
    pc3 = blk(pc, 2).astype(np.float16)
    qc3 = blk(qc, 2).astype(np.float16)
    rb = np.broadcast_to(r32[:, None], (N, MC)).astype(np.float32)
    rb3 = blk(rb, 2).astype(np.float32)
    ph48 = th * float(LLOC)
    rot48 = np.stack(
        [np.cos(ph48), np.sin(ph48), -np.sin(ph48)], axis=1)  # [N, 3]
    r48_3 = np.ascontiguousarray(
        rot48.reshape(2, 128, 3).transpose(1, 0, 2)).astype(np.float32)
    zrot = np.zeros_like(r48_3)

    # skip-path weights: D_hi = fp8(D) as duplicated diag blocks; the
    # residual D_lo*x_hi cross-term is folded into the second x plane on the
    # host (xm = x_lo + (D_lo/D_hi)*x_hi), so one DoubleRow matmul computes
    # D_hi*(x_hi + xm) = D*x to ~1%.  Channels with near-zero D_hi get
    # ratio 0 (their whole skip term is negligible).
    d_hi = Dv.astype(f8).astype(np.float32)
    d_lo = Dv - d_hi
    d_rat = np.where(np.abs(d_hi) > 1e-3, d_lo / np.where(d_hi == 0, 1, d_hi), 0.0)
    dd5 = np.zeros((128, 8, 2, 128), np.float32)
    idx = np.arange(128)
    for ki in range(8):
        for dup in range(2):
            dd5[idx, ki, dup, idx] = d_hi[128 * ki + idx]
    dd5 = dd5.astype(f8)
    gam3 = np.ascontiguousarray(
        d_rat.reshape(8, 128).T)[:, :, None].astype(np.float32)  # [128, 8, 1]

    in_maps = []
    for c in range(NCORE):
        b, h = c // 2, c % 2
        xs = x[b, h * LLOC:(h + 1) * LLOC, :]                     # [LLOC, D]
        xT3 = np.ascontiguousarray(
            xs.T.reshape(8, 128, LLOC).transpose(1, 0, 2))        # [128,8,LLOC]
        x_hi = xT3.astype(f8)
        xh32 = x_hi.astype(np.float32)
        xm = ((xT3 - xh32) + gam3 * xh32).astype(f8)
        xq4 = np.ascontiguousarray(
            np.stack([x_hi, xm], axis=1))                         # [128,2,8,LLOC]
        gm = np.zeros((128, 4), np.float32)
        pm = np.zeros((128, 4), np.float32)
        if h == 0:
            gm[:, b] = 1.0      # first-half core contributes to its pair's group
        pm[:, b] = 1.0          # every core selects its pair's group
        in_maps.append({
            "xq": xq4, "bg8": bg83, "ct": ct3,
            "cost": cs3, "sint": sn3, "rb": rb3, "rpow": rpw3,
            "pctab": pc3, "qctab": qc3,
            "rot48": (r48_3 if h == 1 else zrot),
            "gmask": gm, "pmask": pm, "ddiag": dd5,
        })
    return in_maps


def kernel(**inputs) -> np.ndarray:
    global LAST_RESULTS
    from concourse.bass_utils import run_bass_kernel_spmd

    if "nc" not in _CACHE:
        _CACHE["nc"] = _build()
    nc = _CACHE["nc"]

    in_maps = _prep(inputs)
    trace = os.environ.get("LRU_TRACE", "0") == "1"
    res = run_bass_kernel_spmd(
        nc, in_maps, core_ids=list(range(NCORE)), trace=trace,
        trace_cores=list(range(NCORE)) if trace else None,
        stitch_traces=trace,
    )
    LAST_RESULTS = res

    out = np.empty((B, L, D), np.float32)
    for c in range(NCORE):
        b, h = c // 2, c % 2
        o3 = np.asarray(res.results[c]["outT"])          # [128, 8, LLOC] f16
        o2 = o3.transpose(1, 0, 2).reshape(D, LLOC)      # [D, LLOC]
        out[b, h * LLOC:(h + 1) * LLOC, :] = o2.T.astype(np.float32)
    return out



# revision 2
# speedup vs baseline: 1.0258x; 1.0258x over previous
"""LRU forward on 8 Trainium2 NeuronCores.

Sharding: 8 shards = 4 batches x 2 sequence halves (L_local = 2048).
Per-core dataflow is fully transposed (d_model on SBUF partitions, time on
the free dim):

  input proj   Bu^T = Bg_cat^T @ x^T as fp8e4 DoubleRow matmuls (weights
               pre-scaled x64 on the host to stay in e4m3 normal range; the
               1/64 is folded into the PSUM->SBUF downcast scale)
  scan         complex diagonal recurrence -> rotating frame e^{-i theta t}
               turns it into 4 real per-lane scans (hardware
               tensor_tensor_scan; fp32 multiplier broadcast-AP + internal
               state, fp16 data).  Carry between sequence halves exchanged
               with a pairwise AllReduce and applied as
               g += r^{512m+s+1} * c_hat (r^{512m} folded into per-chunk
               chat scalars); chunk 0's rot-out runs carry-free inside the
               exchange bubble and is corrected afterwards via host P/Q
               tables (rpow x cos/sin) -- those corrections write fp8 h
               directly (TensorScalarPtr has no 2x mode to lose)
  output proj  ys^T = CT_cat^T @ h as fp8 DoubleRow matmuls (CT pre-scaled
               x64, h stored fp8; channel-pair packing puts (re_c0, re_c1)
               and (im_c0, im_c1) in the 2-row DR slot), plus the D*x skip
               path as one fp8 DR matmul per block: diag(64*D_hi) against
               the x_hi plane and a host-merged plane
               xm = x_lo + (D_lo/D_hi)*x_hi

Rotation elementwise work runs on DVE in fp16 (2x perf mode) with the
imaginary-path multiplies (rot-in) and the final re/im combine adds
(rot-out, fp8 out) offloaded to gpsimd; PSUM->SBUF downcasts run on the
scalar (Activation) engine as one 4-bank op per group; tables are SBUF
resident and loaded once.  Phase A uses non-uniform time chunks (small at
both ends) to shorten pipeline fill and the scan->collective tail, and junk
matmuls chained on the exchange arrival keep the tensor engine's p-state
ramped through the collective bubble.  Host side only preprocesses/shards
and reassembles the output.
"""

import os

import numpy as np

B, L, D, N = 4, 4096, 1024, 256
NCORE = 8
LLOC = L // 2          # per-core sequence length
MC = 512               # time chunk (matmul moving free dim)
NMC = LLOC // MC       # 4 chunks
N2 = 2 * N             # stacked re|im channels

_CACHE = {}
LAST_RESULTS = None    # test.py reads exec_time_ns from here


def _build():
    import concourse.bass as bass
    import concourse.mybir as mybir
    import concourse.tile as tile
    from concourse import bacc

    f32 = mybir.dt.float32
    f16 = mybir.dt.float16
    f8 = mybir.dt.float8e4
    DR = mybir.MatmulPerfMode.DoubleRow
    ADD = mybir.AluOpType.add
    SUB = mybir.AluOpType.subtract
    MUL = mybir.AluOpType.mult

    nc = bacc.Bacc("TRN2", target_bir_lowering=False, debug=False, num_devices=NCORE)

    # ---- DRAM I/O (per-core) ----
    # xq holds two fp8 planes of x^T: [:, 0] = fp8(x) and [:, 1] = the
    # merged residual plane xm = (x - hi) + (D_lo/D_hi)*hi.  The input
    # projection reads the hi plane; the skip path contracts both against
    # duplicated diag(64*D_hi) blocks in a single DoubleRow matmul.
    xqd = nc.dram_tensor("xq", [128, 2, 8, LLOC], f8, kind="ExternalInput").ap()
    bg8d = nc.dram_tensor("bg8", [128, 8, N2], f8, kind="ExternalInput").ap()
    ctd = nc.dram_tensor("ct8", [128, 4, D], f8, kind="ExternalInput").ap()
    ddd = nc.dram_tensor("ddiag", [128, 8, 2, 128], f8, kind="ExternalInput").ap()
    csd = nc.dram_tensor("cost", [128, 2, LLOC], f16, kind="ExternalInput").ap()
    snd = nc.dram_tensor("sint", [128, 2, LLOC], f16, kind="ExternalInput").ap()
    rbd = nc.dram_tensor("rb", [128, 2], f32, kind="ExternalInput").ap()
    rpwd = nc.dram_tensor("rpow", [128, 2, MC], f16, kind="ExternalInput").ap()
    rfmd = nc.dram_tensor("rfm", [128, 2, 3], f32, kind="ExternalInput").ap()
    pcd = nc.dram_tensor("pctab", [128, 2, MC], f16, kind="ExternalInput").ap()
    qcd = nc.dram_tensor("qctab", [128, 2, MC], f16, kind="ExternalInput").ap()
    r48d = nc.dram_tensor("rot48", [128, 2, 3], f32, kind="ExternalInput").ap()
    gmd = nc.dram_tensor("gmask", [128, 4], f32, kind="ExternalInput").ap()
    pmd = nc.dram_tensor("pmask", [128, 4], f32, kind="ExternalInput").ap()
    outd = nc.dram_tensor("outT", [128, 8, LLOC], f16, kind="ExternalOutput").ap()

    with tile.TileContext(nc) as tc:
        from contextlib import ExitStack

        with ExitStack() as st:
            cpool = st.enter_context(tc.tile_pool(name="consts", bufs=1))
            xpool = st.enter_context(tc.tile_pool(name="xt", bufs=1))
            gpool = st.enter_context(tc.tile_pool(name="g", bufs=1))
            bpool = st.enter_context(tc.tile_pool(name="bu", bufs=3))
            upool = st.enter_context(tc.tile_pool(name="u", bufs=3))
            hpool = st.enter_context(tc.tile_pool(name="h", bufs=3))
            opool = st.enter_context(tc.tile_pool(name="o", bufs=3))
            ps = st.enter_context(tc.tile_pool(name="ps", bufs=2, space="PSUM"))
            dram = st.enter_context(tc.tile_pool(name="dram", bufs=1, space="DRAM"))

            # ---- SBUF residents; DMA order is pipeline priority order ----
            bg8_sb = cpool.tile([128, 8, N2], f8, tag="bg8", name="bg8")
            xq_sb = xpool.tile([128, 2, 8, LLOC], f8, tag="xq", name="xq")
            cs_sb = cpool.tile([128, 2, LLOC], f16, tag="cs", name="cs")
            sn_sb = cpool.tile([128, 2, LLOC], f16, tag="sn", name="sn")
            rb_sb = cpool.tile([128, 2], f32, tag="rb", name="rb")
            ct_sb = cpool.tile([128, 4, D], f8, tag="ct", name="ct")
            dd_sb = cpool.tile([128, 8, 2, 128], f8, tag="dd", name="dd")
            rpw_sb = cpool.tile([128, 2, MC], f16, tag="rpw", name="rpw")
            rfm_sb = cpool.tile([128, 2, 3], f32, tag="rfm", name="rfm")
            pc_sb = cpool.tile([128, 2, MC], f16, tag="pc", name="pc")
            qc_sb = cpool.tile([128, 2, MC], f16, tag="qc", name="qc")
            r48_sb = cpool.tile([128, 2, 3], f32, tag="r48", name="r48")
            gm_sb = cpool.tile([128, 4], f32, tag="gm", name="gm")
            pm_sb = cpool.tile([128, 4], f32, tag="pm", name="pm")

            # phase-A chunking: small chunks at the ends to shorten pipeline
            # fill and the scan->collective tail latency
            CHA = [(0, 256), (256, 256), (512, 512), (1024, 512),
                   (1536, 384), (1920, 128)]

            nc.sync.dma_start(bg8_sb[:, 0:4, :], bg8d[:, 0:4, :])
            nc.sync.dma_start(bg8_sb[:, 4:8, :], bg8d[:, 4:8, :])
            nc.sync.dma_start(xq_sb[:, 0, :, 0:256], xqd[:, 0, :, 0:256])
            nc.sync.dma_start(cs_sb[:, :, 0:256], csd[:, :, 0:256])
            nc.sync.dma_start(sn_sb[:, :, 0:256], snd[:, :, 0:256])
            nc.sync.dma_start(rb_sb[:], rbd[:, :])
            nc.sync.dma_start(xq_sb[:, 0, :, 256:MC], xqd[:, 0, :, 256:MC])
            nc.sync.dma_start(cs_sb[:, :, 256:2 * MC], csd[:, :, 256:2 * MC])
            nc.sync.dma_start(sn_sb[:, :, 256:2 * MC], snd[:, :, 256:2 * MC])
            nc.sync.dma_start(xq_sb[:, 0, :, MC:2 * MC], xqd[:, 0, :, MC:2 * MC])
            nc.sync.dma_start(gm_sb[:], gmd[:, :])
            nc.sync.dma_start(pm_sb[:], pmd[:, :])
            nc.sync.dma_start(r48_sb[:], r48d[:, :, :])
            nc.sync.dma_start(rfm_sb[:], rfmd[:, :, :])
            nc.sync.dma_start(xq_sb[:, 0, :, 2 * MC:3 * MC],
                              xqd[:, 0, :, 2 * MC:3 * MC])
            nc.sync.dma_start(cs_sb[:, :, 2 * MC:], csd[:, :, 2 * MC:])
            nc.sync.dma_start(xq_sb[:, 0, :, 3 * MC:], xqd[:, 0, :, 3 * MC:])
            nc.sync.dma_start(sn_sb[:, :, 2 * MC:], snd[:, :, 2 * MC:])
            nc.sync.dma_start(dd_sb[:], ddd[:, :, :, :])
            nc.sync.dma_start(pc_sb[:], pcd[:, :, :])
            nc.sync.dma_start(qc_sb[:], qcd[:, :, :])
            nc.sync.dma_start(rpw_sb[:], rpwd[:, :, :])
            # x merged-plane + ct8 load late: their consumers (skip/output
            # matmuls) run inside or after the collective bubble, and an
            # early load would displace phase-A-critical transfers
            nc.sync.dma_start(xq_sb[:, 1, :, :], xqd[:, 1, :, :])
            nc.sync.dma_start(ct_sb[:], ctd[:, :, :])

            # g lanes: 0 = re(c0), 1 = re(c1), 2 = im(c0), 3 = im(c1)
            g_all = gpool.tile([128, 4, LLOC], f16, tag="g", name="g")

            # ---- phase A: input projection + rot-in + scan, per time chunk ----
            # input proj: fp8 DoubleRow matmuls (weights pre-scaled x64 on the
            # host; the 1/64 is folded into the PSUM->SBUF downcast scale).
            # rot-in: the imaginary-path multiplies run on gpsimd to unload DVE.
            for m, (st_, w) in enumerate(CHA):
                ms = slice(st_, st_ + w)
                pt = ps.tile([128, 4, MC], f32, tag="pA", name=f"pbu_{m}")
                for j in range(4):
                    for k2 in range(4):
                        nc.tensor.matmul(
                            pt[:, j, 0:w],
                            bg8_sb[:, 2 * k2:2 * k2 + 2, 128 * j:128 * (j + 1)],
                            xq_sb[:, 0, 2 * k2:2 * k2 + 2, ms],
                            start=(k2 == 0),
                            stop=(k2 == 3),
                            perf_mode=DR,
                        )
                bu = bpool.tile([128, 4, MC], f16, tag="bu", name=f"bu_{m}")
                nc.scalar.mul(bu[:, :, 0:w], pt[:, :, 0:w], 1.0 / 64.0)
                # rot-in as paired 2-free-dim ops: each instruction covers
                # both complex groups; two products feed gpsimd to balance
                csp = cs_sb[:, 0:2, ms]
                snp = sn_sb[:, 0:2, ms]
                p1 = upool.tile([128, 2, MC], f16, tag="p1", name=f"p1_{m}")
                nc.vector.tensor_tensor(p1[:, :, 0:w], bu[:, 0:2, 0:w], csp, MUL)
                p4 = upool.tile([128, 2, MC], f16, tag="p4", name=f"p4_{m}")
                nc.gpsimd.tensor_tensor(p4[:, :, 0:w], bu[:, 0:2, 0:w], snp, MUL)
                p2 = upool.tile([128, 2, MC], f16, tag="p2", name=f"p2_{m}")
                nc.vector.tensor_tensor(p2[:, :, 0:w], bu[:, 2:4, 0:w], snp, MUL)
                p3 = upool.tile([128, 2, MC], f16, tag="p3", name=f"p3_{m}")
                nc.gpsimd.tensor_tensor(p3[:, :, 0:w], bu[:, 2:4, 0:w], csp, MUL)
                u_re = upool.tile([128, 2, MC], f16, tag="ure", name=f"ure_{m}")
                nc.vector.tensor_tensor(u_re[:, :, 0:w], p1[:, :, 0:w],
                                        p2[:, :, 0:w], ADD)
                u_im = upool.tile([128, 2, MC], f16, tag="uim", name=f"uim_{m}")
                nc.vector.tensor_tensor(u_im[:, :, 0:w], p3[:, :, 0:w],
                                        p4[:, :, 0:w], SUB)
                u4 = [u_re[:, 0, :], u_re[:, 1, :], u_im[:, 0, :], u_im[:, 1, :]]
                for tt_ in range(4):
                    init = 0.0 if m == 0 else g_all[:, tt_, st_ - 1:st_]
                    nc.vector.tensor_tensor_scan(
                        g_all[:, tt_, ms],
                        rb_sb[:, tt_ & 1:(tt_ & 1) + 1].broadcast_to([128, w]),
                        u4[tt_][:, 0:w], init, MUL, ADD)

            # ---- phase B: carry exchange (pairwise AllReduce) ----
            stage = cpool.tile([128, 4], f32, tag="stage", name="stage")
            for tt_ in range(4):
                nc.vector.tensor_copy(stage[:, tt_:tt_ + 1],
                                      g_all[:, tt_, LLOC - 1:LLOC])
            # scatter my carry into my pair's 4-column group (zero elsewhere)
            stage16 = cpool.tile([128, 16], f32, tag="stage16", name="stage16")
            for p in range(4):
                nc.vector.tensor_scalar_mul(
                    stage16[:, 4 * p:4 * (p + 1)], stage[:], gm_sb[:, p:p + 1])
            in_cc = dram.tile([128, 16], f32, tag="incc", name="incc")
            out_cc = dram.tile([128, 16], f32, tag="outcc", name="outcc",
                               addr_space="Shared")
            nc.sync.dma_start(in_cc[:], stage16[:])
            if os.environ.get("LRU_NOCC", "0") == "1":
                # collective-free variant for TimelineSim bottleneck analysis
                nc.sync.dma_start(out_cc[:], in_cc[:])
            else:
                nc.gpsimd.collective_compute(
                    "AllReduce",
                    mybir.AluOpType.add,
                    replica_groups=[list(range(NCORE))],
                    ins=[in_cc.opt()],
                    outs=[out_cc.opt()],
                )
            recv16 = cpool.tile([128, 16], f32, tag="recv16", name="recv16")
            nc.sync.dma_start(recv16[:], out_cc[:])

            # skip-path matmuls for chunk 0 run inside the collective bubble
            m0 = slice(0, MC)
            pre_ps = []
            with tc.high_priority():
                for half in range(2):
                    pt = ps.tile([128, 4, MC], f32, tag="pA",
                                 name=f"po{half}_0")
                    pre_ps.append(pt)
                    for dj in range(4):
                        di = 4 * half + dj
                        if di > 0:
                            nc.tensor.matmul(pt[:, dj, :], dd_sb[:, di, :, :],
                                             xq_sb[:, :, di, m0],
                                             start=True, stop=False,
                                             perf_mode=DR)

            # PE warm-up chained on the exchange arrival: junk matmuls into a
            # corner of pre_ps[0] (its real skip matmul below uses start=True
            # and overwrites) keep the tensor engine's p-state ramped through
            # the bubble so the first output matmuls run at full clock
            jsA = cpool.tile([128, 16], f16, tag="jsA", name="jsA")
            nc.scalar.copy(jsA[:], stage16[:])
            for _ in range(34):
                nc.tensor.matmul(pre_ps[0][0:16, 0, 0:128], jsA[:, 0:16],
                                 cs_sb[:, 0, 0:128], start=True, stop=True)
            jsrc = cpool.tile([128, 16], f16, tag="jsrc", name="jsrc")
            nc.scalar.copy(jsrc[:], recv16[:])
            for _ in range(50):
                nc.tensor.matmul(pre_ps[0][0:16, 0, 0:128], jsrc[:, 0:16],
                                 cs_sb[:, 0, 0:128], start=True, stop=True)
            nc.tensor.matmul(pre_ps[0][:, 0, :], dd_sb[:, 0, :, :],
                             xq_sb[:, :, 0, m0],
                             start=True, stop=False, perf_mode=DR)

            # chunk-0 carry-free rot-out also runs inside the bubble; the
            # carry correction lands later via the P/Q tables (rpow folded
            # with cos/sin on the host) and per-lane chat scalars.
            # wait-hint: schedule these AFTER the exchange staging ops above
            # (their data deps are ready much earlier and the scheduler would
            # otherwise run them first, delaying the exchange).
            tc.tile_set_cur_wait(0.040)
            h0 = hpool.tile([128, 4, MC], f16, tag="h0", name="h0_b")
            csm0 = cs_sb[:, 0:2, m0]
            snm0 = sn_sb[:, 0:2, m0]
            t1 = hpool.tile([128, 2, MC], f16, tag="tmpC", name="tE_b")
            nc.vector.tensor_tensor(t1[:], g_all[:, 0:2, m0], csm0, MUL)
            t2 = hpool.tile([128, 2, MC], f16, tag="tmpD", name="tF_b")
            nc.vector.tensor_tensor(t2[:], g_all[:, 2:4, m0], snm0, MUL)
            nc.vector.tensor_tensor(h0[:, 0:2, :], t1[:], t2[:], SUB)
            t3 = hpool.tile([128, 2, MC], f16, tag="tmpC", name="tG_b")
            nc.vector.tensor_tensor(t3[:], g_all[:, 2:4, m0], csm0, MUL)
            t4 = hpool.tile([128, 2, MC], f16, tag="tmpD", name="tH_b")
            nc.vector.tensor_tensor(t4[:], g_all[:, 0:2, m0], snm0, MUL)
            nc.vector.tensor_tensor(h0[:, 2:4, :], t3[:], t4[:], ADD)

            # select my pair's group: recv = sum_p recv16[:, 4p:4p+4] * pm[:, p]
            recv = cpool.tile([128, 4], f32, tag="recv", name="recv")
            nc.vector.tensor_scalar_mul(recv[:], recv16[:, 0:4], pm_sb[:, 0:1])
            for p in range(1, 4):
                nc.vector.scalar_tensor_tensor(
                    recv[:], recv16[:, 4 * p:4 * (p + 1)], pm_sb[:, p:p + 1],
                    recv[:], MUL, ADD)

            # chat = rot48 * recv (per-lane complex rotation), via STT pairs;
            # cols 4,5 hold -chat_im for the chunk-0 P/Q correction
            chat = cpool.tile([128, 6], f32, tag="chat", name="chat")
            tca = cpool.tile([128, 1], f32, tag="tca", name="tca")
            tcb = cpool.tile([128, 1], f32, tag="tcb", name="tcb")
            for c in range(2):
                c48 = r48_sb[:, c, 0:1]
                s48 = r48_sb[:, c, 1:2]
                ns48 = r48_sb[:, c, 2:3]
                # chat_re = recv_re*cos48 - recv_im*sin48
                nc.vector.tensor_tensor(tca[:], recv[:, c:c + 1], c48, MUL)
                nc.vector.scalar_tensor_tensor(
                    chat[:, c:c + 1], recv[:, 2 + c:3 + c], ns48, tca[:], MUL, ADD)
                # chat_im = recv_im*cos48 + recv_re*sin48
                nc.vector.tensor_tensor(tcb[:], recv[:, 2 + c:3 + c], c48, MUL)
                nc.vector.scalar_tensor_tensor(
                    chat[:, 2 + c:3 + c], recv[:, c:c + 1], s48, tcb[:], MUL, ADD)
                nc.vector.tensor_scalar_mul(
                    chat[:, 4 + c:5 + c], chat[:, 2 + c:3 + c], -1.0)

            # per-chunk chat with the r^{512m} decay folded in (lets the
            # carry fixes reuse the single chunk-0 rpow table)
            chatm = cpool.tile([128, 3, 4], f32, tag="chatm", name="chatm")
            for mi in range(3):
                for c in range(2):
                    nc.vector.tensor_scalar_mul(
                        chatm[:, mi, c:4:2], chat[:, c:4:2],
                        rfm_sb[:, c, mi:mi + 1])

            # ---- phase C: carry fix + rot-out + output projection ----
            # h8 lane order matches ct8 row pairs: (re_c0, re_c1, im_c0,
            # im_c1); DR consumes (0:2) and (2:4) as its two rhs rows.
            # tile_wait_until: scheduler-only hint that phase C becomes ready
            # late (after the carry exchange) so ready-early work (the
            # skip-path matmuls) is ordered ahead of it in the engine streams.
            tc.tile_set_cur_wait(0.045)

            def fix(mm, tt_):
                msf = slice(mm * MC, (mm + 1) * MC)
                nc.vector.scalar_tensor_tensor(
                    g_all[:, tt_, msf],
                    rpw_sb[:, tt_ & 1, :],
                    chatm[:, mm - 1, tt_:tt_ + 1],
                    g_all[:, tt_, msf],
                    MUL,
                    ADD,
                )

            for m in range(NMC):
                ms = slice(m * MC, (m + 1) * MC)
                h8 = hpool.tile([128, 4, MC], f8, tag="h8", name=f"h8_{m}")

                if m == 0:
                    # apply the carry to the pre-computed local rot-out:
                    # h_re += chat_re*P - chat_im*Q ; h_im += chat_im*P + chat_re*Q
                    # second STT of each pair writes the fp8 h plane directly;
                    # chunk 1's carry fixes are interleaved between the
                    # correction pairs so its rot-out ladder starts before
                    # all of chunk 0's corrections retire on DVE
                    for comp, pcol, qcol, pre in ((0, 0, 4, 0), (1, 1, 5, 2),
                                                  (2, 2, 0, 1), (3, 3, 1, 3)):
                        c = comp & 1
                        nc.vector.scalar_tensor_tensor(
                            h0[:, comp, :], pc_sb[:, c, :], chat[:, pcol:pcol + 1],
                            h0[:, comp, :], MUL, ADD)
                        nc.vector.scalar_tensor_tensor(
                            h8[:, comp, :], qc_sb[:, c, :], chat[:, qcol:qcol + 1],
                            h0[:, comp, :], MUL, ADD)
                        fix(1, pre)
                else:
                    # carry fix (STT only runs on DVE; the compiler rejects
                    # it on the gpsimd engine); chunk 1's were issued above
                    if m > 1:
                        for tt_ in (0, 2, 1, 3):
                            fix(m, tt_)
                    csm = cs_sb[:, 0:2, ms]
                    snm = sn_sb[:, 0:2, ms]
                    t1 = hpool.tile([128, 2, MC], f16, tag="tmpC", name=f"tE_{m}")
                    nc.vector.tensor_tensor(t1[:], g_all[:, 0:2, ms], csm, MUL)
                    t2 = hpool.tile([128, 2, MC], f16, tag="tmpD", name=f"tF_{m}")
                    nc.vector.tensor_tensor(t2[:], g_all[:, 2:4, ms], snm, MUL)
                    # final re/im combines write fp8 h on gpsimd (frees DVE)
                    nc.gpsimd.tensor_tensor(h8[:, 0:2, :], t1[:], t2[:], SUB)
                    t3 = hpool.tile([128, 2, MC], f16, tag="tmpC", name=f"tG_{m}")
                    nc.vector.tensor_tensor(t3[:], g_all[:, 2:4, ms], csm, MUL)
                    t4 = hpool.tile([128, 2, MC], f16, tag="tmpD", name=f"tH_{m}")
                    nc.vector.tensor_tensor(t4[:], g_all[:, 0:2, ms], snm, MUL)
                    nc.gpsimd.tensor_tensor(h8[:, 2:4, :], t3[:], t4[:], ADD)

                ot = opool.tile([128, 8, MC], f16, tag="ot", name=f"ot_{m}")
                for half in range(2):
                    if m == 0:
                        pt = pre_ps[half]
                    else:
                        pt = ps.tile([128, 4, MC], f32, tag="pA",
                                     name=f"po{half}_{m}")
                    for dj in range(4):
                        di = 4 * half + dj
                        if m > 0:
                            nc.tensor.matmul(pt[:, dj, :], dd_sb[:, di, :, :],
                                             xq_sb[:, :, di, ms],
                                             start=True, stop=False,
                                             perf_mode=DR)
                        for pair in range(2):
                            nc.tensor.matmul(
                                pt[:, dj, :],
                                ct_sb[:, 2 * pair:2 * pair + 2,
                                      128 * di:128 * (di + 1)],
                                h8[:, 2 * pair:2 * pair + 2, :],
                                start=False,
                                stop=(pair == 1),
                                perf_mode=DR,
                            )
                    nc.scalar.mul(ot[:, 4 * half:4 * half + 4, :],
                                  pt[:, :, :], 1.0 / 64.0)
                    if m == NMC - 1 and half == 1:
                        nc.sync.dma_start(outd[:, 4:6, ms], ot[:, 4:6, :])
                        nc.sync.dma_start(outd[:, 6:8, ms], ot[:, 6:8, :])
                    else:
                        nc.sync.dma_start(outd[:, 4 * half:4 * half + 4, ms],
                                          ot[:, 4 * half:4 * half + 4, :])

    nc.compile()
    return nc


def _prep(inputs):
    """Host-side parameter prep + sharding. Returns per-core input maps."""
    x = np.asarray(inputs["input_sequence"], np.float32)
    nu_log = np.asarray(inputs["nu_log"], np.float32)
    theta_log = np.asarray(inputs["theta_log"], np.float32)
    B_re = np.asarray(inputs["B_re"], np.float32)
    B_im = np.asarray(inputs["B_im"], np.float32)
    C_re = np.asarray(inputs["C_re"], np.float32)
    C_im = np.asarray(inputs["C_im"], np.float32)
    Dv = np.asarray(inputs["D"], np.float32)

    r32 = np.exp(-np.exp(nu_log, dtype=np.float32), dtype=np.float32)
    th = np.exp(theta_log, dtype=np.float32).astype(np.float64)
    gamma = np.sqrt((1.0 - r32 * r32).astype(np.float32))

    def blk(a, nb):  # [nb*128, F] -> [128, nb, F]
        return np.ascontiguousarray(
            a.reshape(nb, 128, a.shape[-1]).transpose(1, 0, 2))

    import ml_dtypes
    f8 = ml_dtypes.float8_e4m3

    bg = np.concatenate(
        [(gamma[:, None] * B_re).T, (gamma[:, None] * B_im).T], axis=1)  # [D, 512]
    ct = np.concatenate([C_re.T, -C_im.T], axis=0)                      # [512, D]
    # fp8 weights pre-scaled x64 to stay in e4m3 normal range; the matmul
    # consumer divides by 64 during the PSUM downcast
    bg83 = blk(bg * 64.0, 8).astype(f8)
    ct3 = blk(ct * 64.0, 4).astype(f8)

    t = np.arange(LLOC, dtype=np.float64)
    ang = th[:, None] * t[None, :]
    cs3 = blk(np.cos(ang), 2).astype(np.float16)
    sn3 = blk(np.sin(ang), 2).astype(np.float16)
    r64 = r32.astype(np.float64)
    s = np.arange(MC, dtype=np.float64)
    # chunk-0 decay table r^(s+1); later chunks reuse it with r^{512m}
    # folded into the per-chunk chat scalars on-device
    rpw0 = r64[:, None] ** (s[None, :] + 1.0)                   # [N, MC]
    rpw3 = blk(rpw0, 2).astype(np.float16)
    rfm = np.stack([r64 ** (MC * (mi + 1.0)) for mi in range(3)],
                   axis=1)                                      # [N, 3]
    rfm3 = np.ascontiguousarray(
        rfm.reshape(2, 128, 3).transpose(1, 0, 2)).astype(np.float32)
    # chunk-0 carry-correction tables: r^(s+1) * {cos,sin}(theta*s)
    pc = rpw0 * np.cos(ang[:, :MC])
    qc = rpw0 * np.sin(ang[:, :MC])
    pc3 = blk(pc, 2).astype(np.float16)
    qc3 = blk(qc, 2).astype(np.float16)
    rb3 = np.ascontiguousarray(
        r32.reshape(2, 128).T).astype(np.float32)               # [128, 2]
    ph48 = th * float(LLOC)
    rot48 = np.stack(
        [np.cos(ph48), np.sin(ph48), -np.sin(ph48)], axis=1)  # [N, 3]
    r48_3 = np.ascontiguousarray(
        rot48.reshape(2, 128, 3).transpose(1, 0, 2)).astype(np.float32)
    zrot = np.zeros_like(r48_3)

    # skip-path weights: D_hi = fp8(D) as duplicated diag blocks scaled x64
    # (|64*D_hi| must stay inside e4m3 range); the residual D_lo*x_hi
    # cross-term is folded into the second x plane on the host
    # (xm = x_lo + (D_lo/D_hi)*x_hi), so one DoubleRow matmul computes
    # D_hi*(x_hi + xm) = D*x to ~1%.  Channels with near-zero D_hi get
    # ratio 0 (their whole skip term is negligible).
    d_hi = Dv.astype(f8).astype(np.float32)
    assert np.abs(d_hi * 64.0).max() < 240.0, "64*D_hi overflows e4m3"
    d_lo = Dv - d_hi
    d_rat = np.where(np.abs(d_hi) > 1e-3, d_lo / np.where(d_hi == 0, 1, d_hi), 0.0)
    dd5 = np.zeros((128, 8, 2, 128), np.float32)
    idx = np.arange(128)
    for ki in range(8):
        for dup in range(2):
            dd5[idx, ki, dup, idx] = 64.0 * d_hi[128 * ki + idx]
    dd5 = dd5.astype(f8)
    gam3 = np.ascontiguousarray(
        d_rat.reshape(8, 128).T)[:, :, None].astype(np.float32)  # [128, 8, 1]

    in_maps = []
    for c in range(NCORE):
        b, h = c // 2, c % 2
        xs = x[b, h * LLOC:(h + 1) * LLOC, :]                     # [LLOC, D]
        xT3 = np.ascontiguousarray(
            xs.T.reshape(8, 128, LLOC).transpose(1, 0, 2))        # [128,8,LLOC]
        x_hi = xT3.astype(f8)
        xh32 = x_hi.astype(np.float32)
        xm = ((xT3 - xh32) + gam3 * xh32).astype(f8)
        xq4 = np.ascontiguousarray(
            np.stack([x_hi, xm], axis=1))                         # [128,2,8,LLOC]
        gm = np.zeros((128, 4), np.float32)
        pm = np.zeros((128, 4), np.float32)
        if h == 0:
            gm[:, b] = 1.0      # first-half core contributes to its pair's group
        pm[:, b] = 1.0          # every core selects its pair's group
        in_maps.append({
            "xq": xq4, "bg8": bg83, "ct8": ct3,
            "cost": cs3, "sint": sn3, "rb": rb3, "rpow": rpw3,
            "rfm": rfm3, "pctab": pc3, "qctab": qc3,
            "rot48": (r48_3 if h == 1 else zrot),
            "gmask": gm, "pmask": pm, "ddiag": dd5,
        })
    return in_maps


def kernel(**inputs) -> np.ndarray:
    global LAST_RESULTS
    from concourse.bass_utils import run_bass_kernel_spmd

    if "nc" not in _CACHE:
        _CACHE["nc"] = _build()
    nc = _CACHE["nc"]

    in_maps = _prep(inputs)
    trace = os.environ.get("LRU_TRACE", "0") == "1"
    res = run_bass_kernel_spmd(
        nc, in_maps, core_ids=list(range(NCORE)), trace=trace,
        trace_cores=list(range(NCORE)) if trace else None,
        stitch_traces=trace,
    )
    LAST_RESULTS = res

    out = np.empty((B, L, D), np.float32)
    for c in range(NCORE):
        b, h = c // 2, c % 2
        o3 = np.asarray(res.results[c]["outT"])          # [128, 8, LLOC] f16
        o2 = o3.transpose(1, 0, 2).reshape(D, LLOC)      # [D, LLOC]
        out[b, h * LLOC:(h + 1) * LLOC, :] = o2.T.astype(np.float32)
    return out
